# revision 8
# baseline (speedup 1.0000x reference)
"""3-layer GAT on 8 Trainium2 NeuronCores (Bass/Tile) — v2.

Same math as the v1 baseline (1D node-parallel, deferred-softmax GAT), but
engineered to minimize program size and input upload, which dominate the
measured warm wall under axon:

  - Hardware loops (tc.For_i_unrolled) over node tiles / edge windows instead
    of python unrolling: ~500 instructions total instead of ~11k (16.5MB BIR).
  - Edge index tables are uploaded compactly ([16, T/16] int16, no x8 host
    replication; bf16 dstf) and kept resident in SBUF across all 3 layers.
  - bf16 everywhere off the accumulation path (x, W, node tables, messages,
    one-hot selectors); PSUM/normalization stay f32.
  - Uniform per-(stream,window) edge padding so every loop iteration is
    identical; pad slots gather row 0 and carry dstf=-1 so their one-hot
    column is all-zero (contributes nothing).

Per layer: transform (h|s|d = z @ Wext, node tiles via hw loop) -> AllGather
of the padded node table -> edge phase (hw loop over 49 dst windows: gather
src rows + dst coefficients, ee = exp(leaky_relu(s+d)), messages = h*ee,
one-hot matmul aggregation into PSUM, PSUM -> acc DRAM) -> normalize
(out = num/den + b, ELU, transpose back into zT).
"""

import sys

import numpy as np

sys.path.insert(0, '/opt/trn_rl_repo')

from contextlib import ExitStack

import ml_dtypes

# Persistent XLA compilation cache: run_bass_kernel_spmd builds a fresh
# jax.jit per call, and each fresh compile re-runs bir_verify_and_optimise +
# generate_dve_tables (~0.4s). With the cache, repeat compiles deserialize.
import jax  # noqa: E402

try:
    jax.config.update('jax_compilation_cache_dir', '/tmp/jax_comp_cache')
    jax.config.update('jax_persistent_cache_min_compile_time_secs', 0)
    jax.config.update('jax_persistent_cache_min_entry_size_bytes', 0)
except Exception:
    pass

from concourse import bacc, mybir, tile  # noqa: E402
from concourse.bass import ds, ts  # noqa: E402
from concourse.bass_utils import run_bass_kernel_spmd  # noqa: E402
from concourse.masks import make_identity  # noqa: E402

F32 = mybir.dt.float32
BF16 = mybir.dt.bfloat16
I16 = mybir.dt.int16
AF = mybir.ActivationFunctionType
ALU = mybir.AluOpType
BF = ml_dtypes.bfloat16


class Cfg:
    def __init__(self, N=50000, F=256, H=8, C=32, OUT=2, NCORES=8, NP=1280,
                 GS=768, UNROLL=1, HWLOOP=True):
        self.N, self.F, self.H, self.C, self.OUT = N, F, H, C, OUT
        self.NCORES = NCORES
        assert N % NCORES == 0 and F % 128 == 0
        self.KH = F // 128
        self.NPC = N // NCORES                 # real nodes per core
        self.NT = (self.NPC + 127) // 128      # node tiles / edge windows
        self.NPCP = self.NT * 128              # padded nodes per core
        self.NG = NCORES * self.NPCP           # padded global nodes
        self.SPLIT = (NCORES // 2) * self.NPCP  # stream boundary (int16 idx)
        assert self.SPLIT <= 32768 and self.NG - self.SPLIT <= 32768
        self.EW = 384                          # l1-2 row elems (768B, bf16)
        self.EW3 = 128                         # l3 row elems (256B)
        self.WC = F + 2 * H                    # transform cols (h|s|d)
        self.WC3 = OUT + 2
        self.MC = F + H                        # aggregated cols (msg|ee)
        self.MC3 = OUT + 1
        self.NP = NP                           # padded edges per (stream,win)
        assert NP % 128 == 0
        self.GS = GS                           # idxs per dma_gather call
        assert GS % 128 == 0
        self.UNROLL = UNROLL
        self.HWLOOP = HWLOOP


def _gather_chunks(cfg):
    """[(idx_off, n_idx), ...] covering NP in <=GS multiples of 128."""
    out, off = [], 0
    while off < cfg.NP:
        n = min(cfg.GS, cfg.NP - off)
        out.append((off, n))
        off += n
    return out


def _amat(att):
    Hh, Cc = att.shape
    A = np.zeros((Hh * Cc, Hh), np.float32)
    for h in range(Hh):
        A[h * Cc:(h + 1) * Cc, h] = att[h]
    return A


def _ext_w(W, a_s, a_d):
    Ws = (W @ _amat(a_s)).astype(np.float32)
    Wd = (W @ _amat(a_d)).astype(np.float32)
    return np.ascontiguousarray(
        np.concatenate([W, Ws, Wd], axis=1)).astype(BF)


def _wrap16(arr):
    return np.ascontiguousarray(arr.reshape(-1, 16).T.astype(np.int16))


def _wrap128(arr):
    return np.ascontiguousarray(arr.reshape(-1, 128).T.astype(np.float32))


def _prepare_edges(cfg, src, dst):
    """Host-side partitioning. Returns (NP, per_core list of dicts)."""
    NC, NPC, NPCP, NT = cfg.NCORES, cfg.NPC, cfg.NPCP, cfg.NT
    core_of = dst // NPC
    dstl = dst - core_of * NPC             # 0..NPC-1 (== padded local row)
    win = dstl // 128
    dstw = dstl - win * 128
    srcp = (src // NPC) * NPCP + (src % NPC)   # padded global row
    stream = (srcp >= cfg.SPLIT).astype(np.int64)

    counts = np.zeros((NC, 2, NT), np.int64)
    np.add.at(counts, (core_of, stream, win), 1)
    NP = int(((max(counts.max(), 1) + 127) // 128) * 128)
    cfg.NP = NP
    T = NT * NP

    per_core = []
    for c in range(NC):
        out = {}
        for s, tag in ((0, 'A'), (1, 'B')):
            src_a = np.zeros(T, np.int64)
            dst_a = np.zeros(T, np.int64)
            dstf_a = -np.ones(T, np.float32)
            m = (core_of == c) & (stream == s)
            ww = win[m]
            order = np.argsort(ww, kind='stable')
            ww = ww[order]
            ss = srcp[m][order] - s * cfg.SPLIT
            dd = dstl[m][order]
            fw = dstw[m][order]
            grp_start = np.searchsorted(ww, np.arange(NT))
            rank = np.arange(len(ww)) - grp_start[ww]
            pos = ww * NP + rank
            src_a[pos] = ss
            dst_a[pos] = fw
            dstf_a[pos] = fw
            out['src' + tag] = _wrap16(src_a)
            out['dstw8' + tag] = np.ascontiguousarray(
                dst_a.reshape(-1, 16).T.astype(np.int8))
            out['dstf8' + tag] = np.ascontiguousarray(
                dstf_a.reshape(-1, 128).T.astype(np.int8))
        blob = np.concatenate([
            out['srcA'].ravel(), out['srcB'].ravel(),
            np.concatenate([out['dstw8A'].ravel(),
                            out['dstw8B'].ravel()]).view(np.int16),
            np.concatenate([out['dstf8A'].ravel(),
                            out['dstf8B'].ravel()]).view(np.int16)])
        per_core.append({'ei': np.ascontiguousarray(blob)[None, :]})
    return NP, per_core


def _build(cfg):
    NC, NT, NP, KH = cfg.NCORES, cfg.NT, cfg.NP, cfg.KH
    F, H, C, OUT = cfg.F, cfg.H, cfg.C, cfg.OUT
    NPC, NPCP, NG = cfg.NPC, cfg.NPCP, cfg.NG
    EW, EW3, WC, WC3, MC, MC3 = (cfg.EW, cfg.EW3, cfg.WC, cfg.WC3, cfg.MC,
                                 cfg.MC3)
    T16 = NT * NP // 16
    T128 = NT * NP // 128
    NS = NP // 128                      # 128-edge slices per (stream, window)
    chunks = _gather_chunks(cfg)

    nc = bacc.Bacc('TRN2', target_bir_lowering=False, debug=False,
                   num_devices=NC)

    # ---- I/O ----
    T = NT * NP
    # ALL inputs are packed into ONE int16 tensor (bitcast views carve out
    # int8/bf16/f32 regions): PJRT-over-axon charges a fixed ~7ms per input
    # array per call, so array count matters as much as bytes.
    # Layout (i16 units): srcA|srcB (16-wrapped int16), dstw8A|dstw8B
    # (16-wrapped int8 within-window offsets), dstf8A|dstf8B (128-wrapped
    # int8), xT (int8 pairs), wb (bf16), fb (f32).
    XOFF = 4 * T
    XLEN = F * NPC // 2
    WOFF = XOFF + XLEN
    WLEN = F * (2 * WC + WC3)
    FOFF = WOFF + WLEN
    NF32 = 128 * NT + 2 * F + OUT
    assert XOFF % 2 == 0 and FOFF % 2 == 0
    ei = nc.dram_tensor('ei', [1, FOFF + 2 * NF32], I16,
                        kind='ExternalInput')
    xT = ei[0:1, XOFF:XOFF + XLEN].bitcast(mybir.dt.int8).rearrange(
        'x (p c) -> (x p) c', p=F)
    wb = ei[0:1, WOFF:WOFF + WLEN].bitcast(BF16).rearrange(
        'x (p c) -> (x p) c', p=F)
    fb = ei[0:1, FOFF:FOFF + 2 * NF32].bitcast(F32)
    out_own = nc.dram_tensor('out_own', [NPCP, OUT], mybir.dt.float16,
                             kind='ExternalOutput')

    # ---- internal DRAM ----
    h_own = nc.dram_tensor('h_own', [NPCP, EW], BF16)
    h_full = nc.dram_tensor('h_full', [NG, EW], BF16, addr_space='Shared')
    h3_own = nc.dram_tensor('h3_own', [NPCP, EW3], BF16)
    h3_full = nc.dram_tensor('h3_full', [NG, EW3], BF16, addr_space='Shared')
    acc_d = nc.dram_tensor('acc', [NPCP, MC], F32)
    acc3_d = nc.dram_tensor('acc3', [NPCP, MC3], F32)

    with tile.TileContext(nc) as tc, ExitStack() as ctx:
        const = ctx.enter_context(tc.tile_pool(name='const', bufs=1))
        sb = ctx.enter_context(tc.tile_pool(name='sb', bufs=2))
        eb = ctx.enter_context(tc.tile_pool(name='eb', bufs=2))
        ps = ctx.enter_context(tc.tile_pool(name='ps', bufs=2, space='PSUM'))
        ps1 = ctx.enter_context(tc.tile_pool(name='ps1', bufs=1,
                                             space='PSUM'))

        # ---- constants / resident tables ----
        iota_t = const.tile([128, 128], BF16)
        nc.gpsimd.iota(iota_t[:], pattern=[[1, 128]], base=0,
                       channel_multiplier=0,
                       allow_small_or_imprecise_dtypes=True)
        ident = const.tile([128, 128], F32)
        make_identity(nc, ident[:])
        b_t = {}
        o1 = 128 * NT
        for name, off, w in (('b1', o1, F), ('b2', o1 + F, F),
                             ('b3', o1 + 2 * F, OUT)):
            t = const.tile([128, w], F32, tag='b_' + name)
            nc.sync.dma_start(out=t[:],
                              in_=fb[:, off:off + w].to_broadcast((128, w)))
            b_t[name] = t
        w_t = {}
        for name, off, w in (('W1', 0, WC), ('W2', WC, WC),
                             ('W3', 2 * WC, WC3)):
            t = const.tile([128, KH, w], BF16, tag='w_' + name)
            for kh in range(KH):
                nc.sync.dma_start(out=t[:, kh, :],
                                  in_=wb[kh * 128:(kh + 1) * 128,
                                         off:off + w])
            w_t[name] = t
        idx_t = {}
        for bi, tag in ((0, 'A'), (1, 'B')):
            t = const.tile([128, T16], I16, tag='src' + tag)
            view = ei[0:1, bi * T:(bi + 1) * T].rearrange(
                'x (p c) -> (x p) c', p=16)
            for k in range(8):
                nc.sync.dma_start(out=t[16 * k:16 * (k + 1), :], in_=view)
            idx_t['src' + tag] = t
        eiw = ei[0:1, 2 * T:3 * T].bitcast(mybir.dt.int8)   # [1, 2T] bytes
        ei8 = ei[0:1, 3 * T:4 * T].bitcast(mybir.dt.int8)   # [1, 2T] bytes
        with tc.tile_pool(name='stg', bufs=1) as stg:
            st = stg.tile([128, T16], mybir.dt.int8, tag='stg8')
            for si, tag in ((0, 'A'), (1, 'B')):
                view = eiw[0:1, si * T:(si + 1) * T].rearrange(
                    'x (p c) -> (x p) c', p=16)
                for k in range(8):
                    nc.sync.dma_start(out=st[16 * k:16 * (k + 1), :],
                                      in_=view)
                t = const.tile([128, T16], I16, tag='dst' + tag)
                nc.vector.tensor_copy(out=t[:], in_=st[:])
                idx_t['dst' + tag] = t
            for si, tag in ((0, 'A'), (1, 'B')):
                nc.sync.dma_start(
                    out=st[:, 0:T128],
                    in_=ei8[0:1, si * T:(si + 1) * T].rearrange(
                        'x (p c) -> (x p) c', p=128))
                t = const.tile([128, T128], BF16, tag='dstf' + tag)
                nc.vector.tensor_copy(out=t[:], in_=st[:, 0:T128])
                idx_t['dstf' + tag] = t
        zT = const.tile([128, KH, NPCP], BF16)
        xs_t = const.tile([128, NT], F32)
        nc.sync.dma_start(out=xs_t[:], in_=fb[:, 0:o1].rearrange(
            'x (p t) -> (x p) t', p=128))

        def loop(n, body, unroll):
            if cfg.HWLOOP and n > unroll:
                tc.For_i_unrolled(0, n, 1, body, max_unroll=unroll)
            else:
                for i in range(n):
                    body(i)

        # ---- phase bodies ----
        def transform_body(i, wt, wc, hout, ew, scaled=False):
            pool = ps if wc > 16 else ps1
            p = pool.tile([128, wc], F32, tag=f'ps_tr{wc}')
            # walrus can't take register offsets on the ldweights operand, so
            # stage the dynamic zT slice into a static tile first.
            zs = sb.tile([128, KH, 128], BF16, tag='sb_zs')
            nc.vector.tensor_copy(out=zs[:], in_=zT[:, :, ts(i, 128)])
            for kh in range(KH):
                nc.tensor.matmul(p[:, :], lhsT=zs[:, kh, :],
                                 rhs=wt[:, kh, :], start=(kh == 0),
                                 stop=(kh == KH - 1))
            ht = sb.tile([128, wc], BF16, tag=f'sb_tr{wc}')
            if scaled:
                # undo the int8 row quantization of x (scale is per node,
                # nodes sit on partitions here)
                nc.vector.tensor_tensor(
                    out=ht[:], in0=p[:],
                    in1=xs_t[:, ts(i, 1)].to_broadcast((128, wc)),
                    op=ALU.mult)
            else:
                nc.vector.tensor_copy(out=ht[:], in_=p[:])
            nc.sync.dma_start(out=hout[ts(i, 128), 0:wc], in_=ht[:])

        def edge_body(w, tblf, dtbl, dcol, ew, mc, hh, cc, s_off, d_off,
                      acc):
            """One destination window: gather, ee, messages, aggregate."""
            pool = ps if mc > 16 else ps1
            p = pool.tile([128, mc], F32, tag=f'ps_agg{mc}')
            mm = []  # (lhsT slice, rhs slice) accumulation chain
            for si, (s, tag) in enumerate(((0, 'A'), (1, 'B'))):
                tbl = (tblf[0:cfg.SPLIT, :] if s == 0 else
                       tblf[cfg.SPLIT:NG, :])
                # one set of edge tiles serves all layers: layer 3 (ew=128)
                # reinterprets the 384-wide rows as 3x128 slices
                hg_t = eb.tile([128, NS, EW], BF16, tag=f'hg{tag}')
                hg = (hg_t if ew == EW else
                      hg_t.rearrange('p n (k e) -> p (n k) e',
                                     k=EW // ew)[:, 0:NS, :])
                dg = eb.tile([128, NS, 128], BF16, tag=f'dg{tag}')
                for off, ni in chunks:
                    i16 = w * (NP // 16) + off // 16
                    sl = off // 128
                    nc.gpsimd.dma_gather(
                        hg[:, sl:sl + ni // 128, :], tbl,
                        idx_t['src' + tag][:, ds(i16, ni // 16)],
                        num_idxs=ni, num_idxs_reg=ni, elem_size=ew)
                    nc.gpsimd.dma_gather(
                        dg[:, sl:sl + ni // 128, :],
                        dtbl[ds(w * 128, 128), dcol:dcol + 128],
                        idx_t['dst' + tag][:, ds(i16, ni // 16)],
                        num_idxs=ni, num_idxs_reg=ni, elem_size=128,
                        elem_step=ew)
                e8_t = eb.tile([128, NS, H], F32, tag=f'e8{tag}')
                e8 = e8_t[:, :, 0:hh]
                nc.vector.tensor_tensor(
                    out=e8[:], in0=hg[:, :, s_off:s_off + hh],
                    in1=dg[:, :, d_off:d_off + hh], op=ALU.add)
                el_t = eb.tile([128, NS, H], F32, tag=f'el{tag}')
                el = el_t[:, :, 0:hh]
                nc.vector.tensor_scalar_mul(el[:], e8[:], 0.2)
                nc.vector.tensor_tensor(out=el[:], in0=el[:], in1=e8[:],
                                        op=ALU.max)
                msg_t = eb.tile([128, NS, MC], BF16, tag=f'msg{tag}')
                msg = msg_t[:, :, 0:mc]
                nc.scalar.activation(msg[:, :, hh * cc:hh * cc + hh], el[:],
                                     AF.Exp)
                nc.vector.tensor_tensor(
                    out=msg[:, :, 0:hh * cc].rearrange(
                        'p n (h c) -> p n h c', h=hh),
                    in0=hg[:, :, 0:hh * cc].rearrange(
                        'p n (h c) -> p n h c', h=hh),
                    in1=msg[:, :, hh * cc:hh * cc + hh][
                        :, :, :, None].to_broadcast((128, NS, hh, cc)),
                    op=ALU.mult)
                sel = eb.tile([128, NS, 128], BF16, tag=f'sel{tag}')
                nc.vector.tensor_tensor(
                    out=sel[:],
                    in0=iota_t[:, None, :].to_broadcast((128, NS, 128)),
                    in1=idx_t['dstf' + tag][:, ts(w, NS), None].to_broadcast(
                        (128, NS, 128)),
                    op=ALU.is_equal)
                for j in range(NS):
                    mm.append((sel[:, j, :], msg[:, j, :]))
            for j, (lh, rh) in enumerate(mm):
                nc.tensor.matmul(p[:, :], lhsT=lh, rhs=rh, start=(j == 0),
                                 stop=(j == len(mm) - 1))
            ac = sb.tile([128, mc], F32, tag=f'ac{mc}')
            nc.vector.tensor_copy(out=ac[:], in_=p[:])
            nc.sync.dma_start(out=acc[ts(w, 128), :], in_=ac[:])

        def normalize_body(i, bt):
            a = sb.tile([128, MC], F32, tag='nrm_a')
            nc.sync.dma_start(out=a[:], in_=acc_d[ts(i, 128), :])
            r = sb.tile([128, H], F32, tag='nrm_r')
            nc.vector.tensor_scalar_add(r[:], a[:, F:F + H], 1e-16)
            rr = sb.tile([128, H], F32, tag='nrm_rr')
            nc.vector.reciprocal(rr[:], r[:])
            z = sb.tile([128, F], F32, tag='nrm_z')
            nc.vector.tensor_tensor(
                out=z[:].rearrange('p (h c) -> p h c', h=H),
                in0=a[:, 0:F].rearrange('p (h c) -> p h c', h=H),
                in1=rr[:, :, None].to_broadcast((128, H, C)), op=ALU.mult)
            nc.vector.tensor_tensor(out=z[:], in0=z[:], in1=bt[:],
                                    op=ALU.add)
            # ELU: max(z,0) + exp(min(z,0)) - 1
            zn = sb.tile([128, F], F32, tag='nrm_zn')
            nc.vector.tensor_scalar_min(zn[:], z[:], 0.0)
            en = sb.tile([128, F], F32, tag='nrm_en')
            nc.scalar.activation(en[:], zn[:], AF.Exp)
            nc.vector.tensor_scalar_add(en[:], en[:], -1.0)
            nc.vector.tensor_scalar_max(z[:], z[:], 0.0)
            nc.vector.tensor_tensor(out=z[:], in0=z[:], in1=en[:], op=ALU.add)
            for kh in range(KH):
                tp = ps1.tile([128, 128], F32, tag='ps_tp')
                nc.tensor.transpose(out=tp[:, :],
                                    in_=z[:, kh * 128:(kh + 1) * 128],
                                    identity=ident[:, :])
                nc.vector.tensor_copy(out=zT[:, kh, ts(i, 128)], in_=tp[:, :])

        def final_body(i):
            a = sb.tile([128, MC3], F32, tag='fo_a')
            nc.sync.dma_start(out=a[:], in_=acc3_d[ts(i, 128), :])
            r = sb.tile([128, 1], F32, tag='fo_r')
            nc.vector.tensor_scalar_add(r[:], a[:, OUT:OUT + 1], 1e-16)
            rr = sb.tile([128, 1], F32, tag='fo_rr')
            nc.vector.reciprocal(rr[:], r[:])
            o = sb.tile([128, OUT], F32, tag='fo_o')
            nc.vector.tensor_tensor(out=o[:], in0=a[:, 0:OUT],
                                    in1=rr[:, :].to_broadcast((128, OUT)),
                                    op=ALU.mult)
            o16 = sb.tile([128, OUT], mybir.dt.float16, tag='fo_o16')
            nc.vector.tensor_tensor(out=o16[:], in0=o[:], in1=b_t['b3'][:],
                                    op=ALU.add)
            nc.sync.dma_start(out=out_own[ts(i, 128), :], in_=o16[:])

        def allgather(src_d, dst_d):
            nc.gpsimd.collective_compute(
                'AllGather', ALU.bypass, ins=[src_d[:, :]], outs=[dst_d[:, :]],
                replica_groups=[list(range(NC))])

        # ---------------- program ----------------
        U = cfg.UNROLL
        with nc.named_scope('tr1'):
            # Zero-fill node tables once: the pad columns (WC:EW) are never
            # consumed, but stale DRAM would trip the sim's finite check.
            zpad = const.tile([128, EW], BF16)
            nc.vector.memset(zpad[:], 0)
            nc.sync.dma_start(
                out=h_own[:, :].rearrange('(t p) e -> p t e', p=128),
                in_=zpad[:, None, :].to_broadcast((128, NT, EW)))
            nc.sync.dma_start(
                out=h3_own[:, :].rearrange('(t p) e -> p t e', p=128),
                in_=zpad[:, None, 0:EW3].to_broadcast((128, NT, EW3)))
            xi8 = const.tile([128, KH, NPC], mybir.dt.int8)
            for kh in range(KH):
                nc.sync.dma_start(out=xi8[:, kh, :],
                                  in_=xT[kh * 128:(kh + 1) * 128, :])
            nc.vector.tensor_copy(out=zT[:, :, 0:NPC], in_=xi8[:])
            if NPCP > NPC:
                nc.vector.memset(zT[:, :, NPC:NPCP], 0)
            loop(NT, lambda i: transform_body(i, w_t['W1'], WC, h_own, EW,
                                              scaled=True), U)
        with nc.named_scope('ag1'):
            allgather(h_own, h_full)
        with nc.named_scope('edges1'):
            loop(NT, lambda w: edge_body(
                w, h_full, h_own, F, EW, MC, H, C, F, H, acc_d), U)
        with nc.named_scope('tr2'):
            loop(NT, lambda i: normalize_body(i, b_t['b1']), U)
            loop(NT, lambda i: transform_body(i, w_t['W2'], WC, h_own, EW), U)
        with nc.named_scope('ag2'):
            allgather(h_own, h_full)
        with nc.named_scope('edges2'):
            loop(NT, lambda w: edge_body(
                w, h_full, h_own, F, EW, MC, H, C, F, H, acc_d), U)
        with nc.named_scope('tr3'):
            loop(NT, lambda i: normalize_body(i, b_t['b2']), U)
            loop(NT, lambda i: transform_body(i, w_t['W3'], WC3, h3_own, EW3),
                 U)
        with nc.named_scope('ag3'):
            allgather(h3_own, h3_full)
        with nc.named_scope('edges3'):
            loop(NT, lambda w: edge_body(
                w, h3_full, h3_own, 0, EW3, MC3, 1, OUT, OUT, OUT + 1,
                acc3_d), U)
        with nc.named_scope('fin'):
            loop(NT, final_body, U)

    nc.compile()
    # The module is immutable from here on; memoize its serialization so the
    # per-call jax lowering (which re-serializes the BIR every fresh jit)
    # doesn't redo ~20ms of json+zstd work each run.
    _json = nc.to_json_bytes()
    nc.to_json_bytes = lambda: _json
    return nc


def prepare_all(cfg, x, edge_index, W1, att_src1, att_dst1, b1,
                W2, att_src2, att_dst2, b2, W3, att_src3, att_dst3, b3):
    N = cfg.N
    src = np.concatenate([np.asarray(edge_index[0]),
                          np.arange(N)]).astype(np.int64)
    dst = np.concatenate([np.asarray(edge_index[1]),
                          np.arange(N)]).astype(np.int64)
    NP, per_core = _prepare_edges(cfg, src, dst)
    W1e_ = _ext_w(np.asarray(W1, np.float32), np.asarray(att_src1, np.float32),
                  np.asarray(att_dst1, np.float32))
    W2e_ = _ext_w(np.asarray(W2, np.float32), np.asarray(att_src2, np.float32),
                  np.asarray(att_dst2, np.float32))
    W3e_ = _ext_w(np.asarray(W3, np.float32), np.asarray(att_src3, np.float32),
                  np.asarray(att_dst3, np.float32))
    x = np.asarray(x, np.float32)
    wb_ = np.ascontiguousarray(np.concatenate([W1e_, W2e_, W3e_], axis=1))
    wb16 = wb_.view(np.int16).ravel()
    in_maps = []
    for c in range(cfg.NCORES):
        xc = x[c * cfg.NPC:(c + 1) * cfg.NPC]
        sc = np.maximum(np.abs(xc).max(axis=1), 1e-30) / 127.0   # [NPC]
        xq = np.clip(np.round(xc / sc[:, None]), -127, 127).astype(np.int8)
        scp = np.ones(cfg.NPCP, np.float32)
        scp[:cfg.NPC] = sc
        xs_flat = scp.reshape(cfg.NT, 128).T.astype(np.float32).ravel()
        fb_ = np.concatenate([
            xs_flat,
            np.asarray(b1).ravel().astype(np.float32),
            np.asarray(b2).ravel().astype(np.float32),
            np.asarray(b3).ravel().astype(np.float32)]).astype(np.float32)
        blob = np.concatenate([
            per_core[c]['ei'].ravel(),
            np.ascontiguousarray(xq.T).ravel().view(np.int16),
            wb16,
            fb_.view(np.int16)])
        in_maps.append({'ei': np.ascontiguousarray(blob)[None, :]})
    return NP, in_maps


_CACHE = {}
LAST_RESULT = None
LAST_RUN = None


def run_again():
    import time
    nc, in_maps, ncores = LAST_RUN
    t0 = time.monotonic()
    run_bass_kernel_spmd(nc, in_maps, core_ids=list(range(ncores)))
    return time.monotonic() - t0


def kernel(x, edge_index, W1, att_src1, att_dst1, b1, W2, att_src2, att_dst2,
           b2, W3, att_src3, att_dst3, b3):
    global LAST_RESULT, LAST_RUN
    x = np.asarray(x)
    edge_index = np.asarray(edge_index)
    cfg = Cfg(N=x.shape[0], F=x.shape[1], H=np.asarray(att_src1).shape[0],
              C=np.asarray(att_src1).shape[1], OUT=np.asarray(W3).shape[1])
    NP, in_maps = prepare_all(cfg, x, edge_index, W1, att_src1, att_dst1,
                              b1, W2, att_src2, att_dst2, b2, W3, att_src3,
                              att_dst3, b3)
    key = (cfg.N, cfg.F, NP)
    if key not in _CACHE:
        _CACHE[key] = _build(cfg)
    nc = _CACHE[key]
    LAST_RUN = (nc, in_maps, cfg.NCORES)
    res = run_bass_kernel_spmd(nc, in_maps, core_ids=list(range(cfg.NCORES)))
    LAST_RESULT = res
    out = np.concatenate([res.results[c]['out_own'][:cfg.NPC]
                          for c in range(cfg.NCORES)], axis=0)
    return out.astype(np.float32)
